# revision 22
# baseline (speedup 1.0000x reference)
"""GCN AutoEncoder (6-layer, BN+ReLU) on 8 Trainium2 NeuronCores.

Strategy (dst-sharded graph parallel):
  - nodes partitioned contiguously: core i owns rows [i*NPC, (i+1)*NPC)
  - self-loops materialized as ordinary edges (src=dst)
  - per layer: local transform t = dis * (bn_fold(v) @ W) in bf16, PSUM
    drained by the scalar engine with the per-node dis scale
    -> AllGather t -> h_all table in DRAM (256B rows, bf16 x 128)
    -> batched gathers via the gpsimd dma_gather custom op (int16 indices;
       the 50176-row table is split into two 25088-row halves so indices
       fit int16, with each tile's edge slots partitioned by half)
    -> one batched selection-matrix build per group on DVE
       S[e, c*128+d] = (dst_rel[e,c]==d), then per 128-edge chunk a PE
       matmul M.T @ S accumulated in PSUM per 128-dst tile
    -> epilogue: v = relu(dis*agg + b) on ACT with accum_out giving the
       BN sum for free; sum-of-squares via one whole-layer DVE pass
    -> BN folded into next W via stats AllReduce
Edge slots (pad slots gather row 0 with dst_rel=200 -> S row of zeros),
chunk counts equalized across cores so the SPMD instruction stream is
identical. All heavy per-edge work is preprocessed on the host.
"""
import sys

sys.path.insert(0, "/opt/trn_rl_repo")

import numpy as np
import ml_dtypes

import concourse.bass as bass
import concourse.mybir as mybir
import concourse.tile as tile
from concourse import bacc, library_config
from concourse.bass_utils import run_bass_kernel_spmd

F32 = mybir.dt.float32
BF16 = mybir.dt.bfloat16
I32 = mybir.dt.int32
I16 = mybir.dt.int16
AF = mybir.ActivationFunctionType
ALU = mybir.AluOpType

NCORES = 8
P = 128
EW = 128          # table row width (bf16) -> 256B stride for dma_gather
TPG = 2           # dst tiles per gather group
SUBMAX = 8        # max chunks (x128 idxs) per dma_gather call (desc-ring cap:
                  # 1024 idxs -> 65 descs/engine-ring, HW-validated; 1536 fails)
BF = ml_dtypes.bfloat16


def _split_chunks(k):
    """Split k chunks into sub-calls of <= SUBMAX chunks each."""
    nsub = (k + SUBMAX - 1) // SUBMAX
    base = k // nsub
    rem = k % nsub
    out = []
    pos = 0
    for i in range(nsub):
        n = base + (1 if i < rem else 0)
        out.append((pos, n))
        pos += n
    return out


class Cfg:
    def __init__(self, n_nodes=50000, dims=None):
        self.n = n_nodes
        self.dims = dims or [(88, 70), (70, 60), (60, 50), (50, 60), (60, 70), (70, 88)]
        self.relu = [True, True, False, True, True, False]
        self.bn = [True, True, False, True, True, False]
        self.npc = self.n // NCORES
        assert self.npc * NCORES == self.n
        self.ntiles = (self.npc + P - 1) // P
        self.m_last = self.npc - (self.ntiles - 1) * P
        self.tpc = self.ntiles * P          # table rows per core
        self.half_rows = (NCORES // 2) * self.tpc
        self.eps = 1e-5


def _wrap16(idx):
    """dma_gather index layout: idx[i] at [i%16, i//16], replicated to 128."""
    n = len(idx)
    w = np.zeros((16, n // 16), dtype=np.int16)
    w[np.arange(n) % 16, np.arange(n) // 16] = idx
    return np.tile(w, (8, 1))


def make_groups(ntiles):
    out = []
    for t0 in range(0, ntiles, TPG):
        out.append(list(range(t0, min(t0 + TPG, ntiles))))
    return out


def preprocess(cfg, x, edge_index):
    n, npc, ntiles = cfg.n, cfg.npc, cfg.ntiles
    src_e = np.asarray(edge_index[0], dtype=np.int64).astype(np.int32)
    dst_e = np.asarray(edge_index[1], dtype=np.int64).astype(np.int32)
    deg = np.bincount(dst_e, minlength=n).astype(np.float32) + 1.0
    dis = 1.0 / np.sqrt(deg)

    loop = np.arange(n, dtype=np.int32)
    src = np.concatenate([src_e, loop])
    dst = np.concatenate([dst_e, loop])

    core_of = dst // npc
    tile_of = (dst % npc) // P
    half_of = (src // npc) // (NCORES // 2)

    # chunk counts per (tile, half), equalized across cores
    counts = np.zeros((NCORES, ntiles, 2), dtype=np.int64)
    np.add.at(counts, (core_of, tile_of, half_of), 1)
    cth = np.maximum(1, np.ceil(counts.max(axis=0) / P).astype(np.int64))  # [ntiles, 2]

    groups = make_groups(ntiles)
    # global chunk order: per group: [A-chunks of each tile, then B-chunks]
    # chunk_start[t][h] = global chunk index of (tile t, half h)'s first chunk
    chunk_start = np.zeros((ntiles, 2), dtype=np.int64)
    # per-half global gather order: per group, contiguous run of that half's chunks
    acall_start = []   # per group: (a0, kA, b0, kB) in per-half chunk units
    a_tot = b_tot = 0
    gpos = 0
    chunk_meta = []    # per group: list of tile ids per chunk position
    for tl in groups:
        kA = int(sum(cth[t, 0] for t in tl))
        kB = int(sum(cth[t, 1] for t in tl))
        acall_start.append((a_tot, kA, b_tot, kB))
        meta = []
        for t in tl:
            chunk_start[t, 0] = gpos + len(meta)
            meta += [t] * int(cth[t, 0])
        for t in tl:
            chunk_start[t, 1] = gpos + len(meta)
            meta += [t] * int(cth[t, 1])
        chunk_meta.append(meta)
        gpos += len(meta)
        a_tot += kA
        b_tot += kB
    tot_chunks = gpos

    # slot assignment
    order = np.lexsort((half_of, tile_of, core_of))
    src_s, dst_s = src[order], dst[order]
    core_s, tile_s, half_s = core_of[order], tile_of[order], half_of[order]

    grp = (core_s * ntiles + tile_s) * 2 + half_s
    sort_idx = np.argsort(grp, kind="stable")
    gsorted = grp[sort_idx]
    first = np.r_[True, gsorted[1:] != gsorted[:-1]]
    grp_start = np.flatnonzero(first)
    run_idx = np.arange(len(gsorted)) - np.repeat(
        grp_start, np.diff(np.r_[grp_start, len(gsorted)]))
    pos_in_group = np.empty(len(src_s), dtype=np.int64)
    pos_in_group[sort_idx] = run_idx

    chunk_idx = chunk_start[tile_s, half_s] + pos_in_group // P
    part_idx = pos_in_group % P

    table_row = (src_s // npc) * cfg.tpc + (src_s % npc)
    row_in_half = np.where(half_s == 0, table_row, table_row - cfg.half_rows)
    assert row_in_half.max() < 32768

    # per-core arrays
    dst_rel = np.full((NCORES, P, tot_chunks), 200.0, dtype=np.float32)
    dst_rel[core_s, part_idx, chunk_idx] = (dst_s % npc - tile_s * P).astype(np.float32)

    # gather index lists per half, in call order (slot-major within each call)
    # global chunk -> per-half chunk position
    half_pos = np.zeros((ntiles, 2), dtype=np.int64)
    a_run = b_run = 0
    for tl, (a0, kA, b0, kB) in zip(groups, acall_start):
        for t in tl:
            half_pos[t, 0] = a_run
            a_run += int(cth[t, 0])
        for t in tl:
            half_pos[t, 1] = b_run
            b_run += int(cth[t, 1])
    idxA = np.zeros((NCORES, a_tot * P), dtype=np.int16)
    idxB = np.zeros((NCORES, b_tot * P), dtype=np.int16)
    hchunk = half_pos[tile_s, half_s] + pos_in_group // P
    li = hchunk * P + part_idx
    selA = half_s == 0
    idxA[core_s[selA], li[selA]] = row_in_half[selA]
    idxB[core_s[~selA], li[~selA]] = row_in_half[~selA]

    xs = np.asarray(x, dtype=np.float32)
    f_in0 = xs.shape[1]
    xT = np.zeros((NCORES, f_in0, cfg.tpc), dtype=BF)
    dis_col = np.zeros((NCORES, P, ntiles), dtype=np.float32)
    fmax = max(fo for _, fo in cfg.dims)
    dis_rep = np.zeros((NCORES, fmax, cfg.tpc), dtype=np.float32)
    for i in range(NCORES):
        sl = slice(i * npc, (i + 1) * npc)
        xT[i, :, :npc] = xs[sl].T.astype(BF)
        d = dis[sl]
        dis_col[i, : len(d) - (ntiles - 1) * P, ntiles - 1] = d[(ntiles - 1) * P:]
        for t in range(ntiles - 1):
            dis_col[i, :, t] = d[t * P:(t + 1) * P]
        dis_rep[i, :, :npc] = d[None, :]

    kmax = max(kA + kB for _, kA, _, kB in acall_start)
    iota_rep = np.tile(np.arange(P, dtype=np.float32), (P, kmax)).astype(BF)

    return dict(
        cth=cth, groups=groups, acall_start=acall_start, chunk_meta=chunk_meta,
        tot_chunks=tot_chunks, a_tot=a_tot, b_tot=b_tot, kmax=kmax,
        dst_rel=dst_rel.astype(BF),
        idxA=np.stack([_wrap16(idxA[i]) for i in range(NCORES)]),
        idxB=np.stack([_wrap16(idxB[i]) for i in range(NCORES)]),
        xT=xT, dis_col=dis_col, dis_rep=dis_rep, iota_rep=iota_rep,
    )


def build_nc(cfg, pre):
    n, npc, ntiles, m_last = cfg.n, cfg.npc, cfg.ntiles, cfg.m_last
    dims = cfg.dims
    tot_chunks = pre["tot_chunks"]
    a_tot, b_tot, kmax = pre["a_tot"], pre["b_tot"], pre["kmax"]
    groups, acall_start, chunk_meta = pre["groups"], pre["acall_start"], pre["chunk_meta"]
    fmax = max(fo for _, fo in dims)
    f_in0 = dims[0][0]
    rg = [list(range(NCORES))]

    nc = bacc.Bacc("TRN2", target_bir_lowering=False, debug=False, num_devices=NCORES)

    xT_e = nc.dram_tensor("xT", [f_in0, cfg.tpc], BF16, kind="ExternalInput")
    ia_e = nc.dram_tensor("idxA", [P, a_tot * 8], I16, kind="ExternalInput")
    ib_e = nc.dram_tensor("idxB", [P, b_tot * 8], I16, kind="ExternalInput")
    drel_e = nc.dram_tensor("dst_rel", [P, tot_chunks], BF16, kind="ExternalInput")
    iota_e = nc.dram_tensor("iota_rep", [P, kmax * P], BF16, kind="ExternalInput")
    dcol_e = nc.dram_tensor("dis_col", [P, ntiles], F32, kind="ExternalInput")
    drep_e = nc.dram_tensor("dis_rep", [fmax, cfg.tpc], F32, kind="ExternalInput")
    b6r_e = nc.dram_tensor("b6_rep", [P, dims[5][1]], F32, kind="ExternalInput")
    w_e, b_e, g_e, be_e = [], [], [], []
    for l, (fi, fo) in enumerate(dims):
        w_e.append(nc.dram_tensor(f"W{l}", [fi, fo], BF16, kind="ExternalInput"))
        b_e.append(nc.dram_tensor(f"b{l}", [fo, 1], F32, kind="ExternalInput"))
        if cfg.bn[l]:
            g_e.append(nc.dram_tensor(f"g{l}", [fo, 1], F32, kind="ExternalInput"))
            be_e.append(nc.dram_tensor(f"be{l}", [fo, 1], F32, kind="ExternalInput"))
        else:
            g_e.append(None)
            be_e.append(None)
    out_e = nc.dram_tensor("out", [cfg.tpc, dims[5][1]], BF16, kind="ExternalOutput")

    with tile.TileContext(nc) as tc:
        with (
            tc.tile_pool(name="const", bufs=1) as cpool,
            tc.tile_pool(name="vt", bufs=2) as vtpool,
            tc.tile_pool(name="tsb", bufs=1) as tpool,
            tc.tile_pool(name="sqb", bufs=1) as qpool,
            tc.tile_pool(name="mg", bufs=3) as mpool,
            tc.tile_pool(name="ssb", bufs=2) as spool,
            tc.tile_pool(name="eps", bufs=4) as epool,
            tc.tile_pool(name="stat", bufs=2) as stpool,
            tc.tile_pool(name="psA", bufs=4, space="PSUM") as psA,
            tc.tile_pool(name="psB", bufs=3, space="PSUM") as psB,
            tc.tile_pool(name="dram", bufs=1, space="DRAM") as dram,
        ):
            nc.gpsimd.load_library(library_config.mlp)

            def load(pool, e, shape, dtype=F32):
                t = pool.tile(shape, dtype, name=f"c_{e.name}")
                nc.sync.dma_start(t[:], e[:])
                return t

            xT_sb = load(cpool, xT_e, [f_in0, cfg.tpc], BF16)
            ia_sb = load(cpool, ia_e, [P, a_tot * 8], I16)
            ib_sb = load(cpool, ib_e, [P, b_tot * 8], I16)
            drel_sb = load(cpool, drel_e, [P, tot_chunks], BF16)
            iota_sb = load(cpool, iota_e, [P, kmax * P], BF16)
            dcol_sb = load(cpool, dcol_e, [P, ntiles])
            drep_sb = load(cpool, drep_e, [fmax, cfg.tpc])
            b6r_sb = load(cpool, b6r_e, [P, dims[5][1]])
            w_sb = [load(cpool, w_e[l], [dims[l][0], dims[l][1]], BF16) for l in range(6)]
            b_sb = [load(cpool, b_e[l], [dims[l][1], 1]) for l in range(6)]
            g_sb = [load(cpool, g_e[l], [dims[l][1], 1]) if cfg.bn[l] else None for l in range(6)]
            be_sb = [load(cpool, be_e[l], [dims[l][1], 1]) if cfg.bn[l] else None for l in range(6)]

            ag_in = [dram.tile([cfg.tpc, EW], BF16, tag=f"agin{l}", name=f"agin{l}") for l in range(6)]
            ag_out = [dram.tile([NCORES * cfg.tpc, EW], BF16, tag=f"agout{l}", name=f"agout{l}") for l in range(6)]
            ar_in = [dram.tile([dims[l][1], 2], F32, tag=f"arin{l}", name=f"arin{l}") if cfg.bn[l] else None for l in range(6)]
            ar_out = [dram.tile([dims[l][1], 2], F32, tag=f"arout{l}", name=f"arout{l}") if cfg.bn[l] else None for l in range(6)]

            prev_vT = None
            bn_cur = None

            for l in range(6):
                f_in, f_out = dims[l]
                tile_ms = [P] * (ntiles - 1) + [m_last]

                # ---------- transform: t = dis * (bn(v) @ W), bf16
                t_sb = tpool.tile([P, ntiles * EW], BF16, tag="tsb", name="tsb")
                nc.vector.memset(t_sb[:], 0)
                for t in range(ntiles):
                    m = tile_ms[t]
                    if l == 0:
                        lhsT = xT_sb[:f_in, t * P:t * P + m]
                    else:
                        vbn = epool.tile([fmax, P], BF16, tag="vbn", name="vbn")
                        if bn_cur is not None:
                            gs_c, cv_c = bn_cur
                            nc.scalar.activation(vbn[:f_in, :m],
                                                 prev_vT[:f_in, t * P:t * P + m],
                                                 AF.Identity,
                                                 bias=cv_c[:f_in, 0:1],
                                                 scale=gs_c[:f_in, 0:1])
                        else:
                            nc.scalar.activation(vbn[:f_in, :m],
                                                 prev_vT[:f_in, t * P:t * P + m],
                                                 AF.Identity)
                        lhsT = vbn[:f_in, :m]
                    tps = psB.tile([P, fmax], F32, tag="tps", name="tps")
                    nc.tensor.matmul(tps[:m, :f_out], lhsT=lhsT,
                                     rhs=w_sb[l][:f_in, :f_out], start=True, stop=True)
                    # PSUM drain + per-node dis scale on the scalar engine
                    nc.scalar.activation(t_sb[:m, t * EW:t * EW + f_out],
                                         tps[:m, :f_out], AF.Identity,
                                         scale=dcol_sb[:m, t:t + 1])
                nc.sync.dma_start(
                    ag_in[l][:].rearrange("(t p) f -> p t f", p=P),
                    t_sb[:].rearrange("p (t f) -> p t f", f=EW))

                # ---------- AllGather
                nc.gpsimd.collective_compute(
                    "AllGather", ALU.bypass,
                    ins=[ag_in[l][:].opt()],
                    outs=[ag_out[l][:].opt()],
                    replica_groups=rg,
                )

                # ---------- aggregation
                if cfg.bn[l]:
                    ssum = stpool.tile([f_out, ntiles], F32, tag="ssum", name="ssum")
                if l < 5:
                    vT = vtpool.tile([fmax, ntiles * P], F32, tag="vt", name="vt")
                else:
                    out_full = tpool.tile([P, ntiles * dims[5][1]], BF16, tag="ofull", name="ofull")
                    if m_last < P:
                        nc.vector.memset(
                            out_full[:, (ntiles - 1) * dims[5][1]:ntiles * dims[5][1]], 0)

                gc0 = 0
                for gi, tl in enumerate(groups):
                    a0, kA, b0, kB = acall_start[gi]
                    k = kA + kB
                    meta = chunk_meta[gi]
                    mg_g = mpool.tile([P, kmax * EW], BF16, tag="mg", name="mg")
                    for h0, nch in _split_chunks(kA):
                        nc.gpsimd.dma_gather(
                            mg_g[:, h0 * EW:(h0 + nch) * EW].rearrange("p (c f) -> p c f", f=EW),
                            ag_out[l][:cfg.half_rows, :],
                            ia_sb[:, (a0 + h0) * 8:(a0 + h0 + nch) * 8],
                            nch * P, nch * P, EW)
                    for h0, nch in _split_chunks(kB):
                        nc.gpsimd.dma_gather(
                            mg_g[:, (kA + h0) * EW:(kA + h0 + nch) * EW].rearrange("p (c f) -> p c f", f=EW),
                            ag_out[l][cfg.half_rows:, :],
                            ib_sb[:, (b0 + h0) * 8:(b0 + h0 + nch) * 8],
                            nch * P, nch * P, EW)
                    s_g = spool.tile([P, kmax * P], BF16, tag="ssb", name="ssb")
                    nc.vector.tensor_tensor(
                        out=s_g[:, :k * P].rearrange("p (c d) -> p c d", d=P),
                        in0=drel_sb[:, gc0:gc0 + k].unsqueeze(2).broadcast_to([P, k, P]),
                        in1=iota_sb[:, :k * P].rearrange("p (c d) -> p c d", d=P),
                        op=ALU.is_equal)

                    aggs = {}
                    first_pos = {t: meta.index(t) for t in tl}
                    last_pos = {t: len(meta) - 1 - meta[::-1].index(t) for t in tl}
                    for j, t in enumerate(meta):
                        m = tile_ms[t]
                        if t not in aggs:
                            aggs[t] = psA.tile([P, P], F32, tag="agg", name="agg")
                        agg = aggs[t]
                        mg_sl = mg_g[:, j * EW:j * EW + f_out]
                        s_sl = s_g[:, j * P:j * P + m]
                        if l < 5:
                            nc.tensor.matmul(agg[:f_out, :m], lhsT=mg_sl, rhs=s_sl,
                                             start=(j == first_pos[t]),
                                             stop=(j == last_pos[t]))
                        else:
                            nc.tensor.matmul(agg[:m, :f_out], lhsT=s_sl, rhs=mg_sl,
                                             start=(j == first_pos[t]),
                                             stop=(j == last_pos[t]))
                        if j == last_pos[t]:
                            if l < 5:
                                tmp2 = epool.tile([fmax, P], F32, tag="etmp2", name="etmp2")
                                nc.vector.tensor_tensor(
                                    out=tmp2[:f_out, :m], in0=agg[:f_out, :m],
                                    in1=drep_sb[:f_out, t * P:t * P + m], op=ALU.mult)
                                vsl = vT[:f_out, t * P:t * P + m]
                                if cfg.bn[l]:
                                    nc.scalar.activation(
                                        vsl, tmp2[:f_out, :m],
                                        AF.Relu if cfg.relu[l] else AF.Identity,
                                        bias=b_sb[l][:f_out, 0:1],
                                        accum_out=ssum[:f_out, t:t + 1])
                                else:
                                    nc.scalar.activation(
                                        vsl, tmp2[:f_out, :m],
                                        AF.Relu if cfg.relu[l] else AF.Identity,
                                        bias=b_sb[l][:f_out, 0:1])
                            else:
                                ftmp = epool.tile([P, fmax], F32, tag="ftmp", name="ftmp")
                                nc.scalar.activation(ftmp[:m, :f_out], agg[:m, :f_out],
                                                     AF.Identity,
                                                     scale=dcol_sb[:m, t:t + 1])
                                osl = out_full[:m, t * f_out:(t + 1) * f_out]
                                nc.vector.tensor_tensor(out=osl, in0=ftmp[:m, :f_out],
                                                        in1=b6r_sb[:m, :f_out], op=ALU.add)
                    gc0 += k

                if l == 5:
                    nc.sync.dma_start(
                        out_e[:].rearrange("(t p) f -> p t f", p=P),
                        out_full[:].rearrange("p (t f) -> p t f", f=dims[5][1]))

                # ---------- stats AllReduce + fold into next W
                if l < 5:
                    if cfg.bn[l]:
                        pack = stpool.tile([f_out, 2], F32, tag="pack", name="pack")
                        nc.vector.tensor_reduce(out=pack[:f_out, 0:1],
                                                in_=ssum[:f_out, :ntiles],
                                                axis=mybir.AxisListType.X, op=ALU.add)
                        sqb = qpool.tile([fmax, ntiles * P], BF16, tag="sqb", name="sqb")
                        nc.vector.tensor_tensor(out=sqb[:f_out, :npc],
                                                in0=vT[:f_out, :npc],
                                                in1=vT[:f_out, :npc], op=ALU.mult)
                        nc.vector.tensor_reduce(out=pack[:f_out, 1:2],
                                                in_=sqb[:f_out, :npc],
                                                axis=mybir.AxisListType.X, op=ALU.add)
                        nc.sync.dma_start(ar_in[l][:], pack[:f_out, :])
                        nc.gpsimd.collective_compute(
                            "AllReduce", ALU.add,
                            ins=[ar_in[l][:].opt()],
                            outs=[ar_out[l][:].opt()],
                            replica_groups=rg,
                        )
                        st = stpool.tile([f_out, 2], F32, tag="st", name="st")
                        nc.sync.dma_start(st[:f_out, :], ar_out[l][:])
                        mu = stpool.tile([f_out, 1], F32, tag="mu", name="mu")
                        nc.vector.tensor_scalar_mul(mu[:f_out, :], st[:f_out, 0:1], 1.0 / cfg.n)
                        msq = stpool.tile([f_out, 1], F32, tag="msq", name="msq")
                        nc.vector.tensor_scalar_mul(msq[:f_out, :], st[:f_out, 1:2], 1.0 / cfg.n)
                        var = stpool.tile([f_out, 1], F32, tag="var", name="var")
                        nc.vector.tensor_tensor(out=var[:f_out, :], in0=mu[:f_out, :],
                                                in1=mu[:f_out, :], op=ALU.mult)
                        nc.vector.tensor_tensor(out=var[:f_out, :], in0=msq[:f_out, :],
                                                in1=var[:f_out, :], op=ALU.subtract)
                        nc.vector.tensor_scalar_add(var[:f_out, :], var[:f_out, :], cfg.eps)
                        rv = stpool.tile([f_out, 1], F32, tag="rv", name="rv")
                        nc.vector.reciprocal(rv[:f_out, :], var[:f_out, :])
                        rstd = stpool.tile([f_out, 1], F32, tag="rstd", name="rstd")
                        nc.scalar.activation(rstd[:f_out, :], rv[:f_out, :], AF.Sqrt)
                        gs = stpool.tile([f_out, 1], F32, tag="gs", name="gs")
                        nc.vector.tensor_tensor(out=gs[:f_out, :], in0=g_sb[l][:f_out, :],
                                                in1=rstd[:f_out, :], op=ALU.mult)
                        cv = stpool.tile([f_out, 1], F32, tag="cv", name="cv")
                        nc.vector.tensor_tensor(out=cv[:f_out, :], in0=gs[:f_out, :],
                                                in1=mu[:f_out, :], op=ALU.mult)
                        nc.vector.tensor_tensor(out=cv[:f_out, :], in0=be_sb[l][:f_out, :],
                                                in1=cv[:f_out, :], op=ALU.subtract)
                        bn_cur = (gs, cv)
                    else:
                        bn_cur = None
                    prev_vT = vT

    nc.compile()
    return nc


_CACHE = {}


def _get_compiled(cfg, key, pre):
    if key not in _CACHE:
        _CACHE[key] = build_nc(cfg, pre)
    return _CACHE[key]


def make_in_maps(cfg, pre, inputs):
    b6_rep = np.tile(np.asarray(inputs["b6"], dtype=np.float32)[None, :], (P, 1))
    bn_map = {0: "1", 1: "2", 3: "3", 4: "4"}
    in_maps = []
    for i in range(NCORES):
        m = {
            "xT": pre["xT"][i],
            "idxA": pre["idxA"][i],
            "idxB": pre["idxB"][i],
            "dst_rel": pre["dst_rel"][i],
            "iota_rep": pre["iota_rep"],
            "dis_col": pre["dis_col"][i],
            "dis_rep": pre["dis_rep"][i],
            "b6_rep": b6_rep,
        }
        for l in range(6):
            m[f"W{l}"] = np.asarray(inputs[f"W{l+1}"], dtype=np.float32).astype(BF)
            m[f"b{l}"] = np.asarray(inputs[f"b{l+1}"], dtype=np.float32)[:, None]
            if cfg.bn[l]:
                m[f"g{l}"] = np.asarray(inputs[f"g{bn_map[l]}"], dtype=np.float32)[:, None]
                m[f"be{l}"] = np.asarray(inputs[f"be{bn_map[l]}"], dtype=np.float32)[:, None]
        in_maps.append(m)
    return in_maps


def _run(inputs, trace=False):
    cfg = Cfg(n_nodes=int(np.asarray(inputs["x"]).shape[0]))
    x = np.asarray(inputs["x"], dtype=np.float32)
    edge_index = np.asarray(inputs["edge_index"])
    pre = preprocess(cfg, x, edge_index)
    key = (cfg.n, edge_index.shape[1], hash(edge_index.tobytes()))
    nc = _get_compiled(cfg, key, pre)

    in_maps = make_in_maps(cfg, pre, inputs)
    res = run_bass_kernel_spmd(nc, in_maps, core_ids=list(range(NCORES)), trace=trace)
    parts = [res.results[i]["out"][:cfg.npc].astype(np.float32) for i in range(NCORES)]
    out = np.concatenate(parts, axis=0)
    return out, res.exec_time_ns, res


def kernel(**inputs) -> np.ndarray:
    out, _, _ = _run(inputs, trace=False)
    return out


def kernel_traced(**inputs):
    try:
        import trnprof  # noqa: F401  (registers the NTFF profile hook)
    except ImportError:
        pass
    return _run(inputs, trace=True)
